# revision 11
# baseline (speedup 1.0000x reference)
"""Multi-head causal attention (B=4, S=2048, C=1024, H=16, D=64) on 8 trn2 cores.

Sharding: batch x head-half. Core c = (batch b = c//2, head-half hh = c%2).
Each core projects Q/K/V for its 8 heads over the full 2048-token sequence
(no duplicated K/V work), runs causal attention for those heads, and after
each 512-row q-group exchanges the per-head attention outputs O^T with its
pair core via a 2-core AllGather (DRAM bounce). The output projection is
column-split: each core multiplies the full 16-head O^T by its 512-column
slice of Wp, so no partial-sum reduction is needed. The host concatenates
the two column halves per batch.

Per-core pipeline (matmuls bf16, fp32 PSUM):
  P1: DMA bf16 inputs; project K^T [512,2048], Q^T [512,2048] (head-pair
      partition layout), V [seq,8,65] with a ones column per head.
  P2: per q-group qg (512 rows = blocks 2qg,2qg+1), head h, kt-pair g:
      scores S^T via K^T-lhsT matmul, exp on ACT (scale 1/8), causal masks
      on DVE for boundary key tiles, PV accumulation with [V_h | ones]
      giving O^T rows + softmax denominators, reciprocal+normalize.
      After each qg: AllGather O^T slice with pair core.
  P3: per 128-q tile: out[:, own 512 cols] = O^T_all.T @ Wp_cols + bp_cols.
  K/Q/V projection chunks for later q-groups and P3 tiles are interleaved
  into P2 as PE filler work.
"""

import numpy as np

B, S, C, H, D = 4, 2048, 1024, 16, 64
HD = H * D
HL = 512           # head-local hd per core (8 heads x 64)
QB = 256           # q block width
CK = C // 128      # contraction chunks
NCORES = 8

_CACHE = {}


def _build_nc():
    import concourse.bacc as bacc
    import concourse.mybir as mybir
    import concourse.tile as tile

    dt = mybir.dt
    F32, BF = dt.float32, dt.bfloat16
    EXP = mybir.ActivationFunctionType.Exp

    nc = bacc.Bacc(num_swdge_queues=4, num_devices=NCORES)
    xt_d = nc.declare_dram_parameter("xt", [C, S], BF, isOutput=False)
    wk_d = nc.declare_dram_parameter("wk", [C, HL], BF, isOutput=False)
    wv_d = nc.declare_dram_parameter("wv", [C, HL], BF, isOutput=False)
    wq_d = nc.declare_dram_parameter("wq", [C, HL], BF, isOutput=False)
    wp_d = nc.declare_dram_parameter("wp", [HD, HL], BF, isOutput=False)
    bp_d = nc.declare_dram_parameter("bp", [1, HL], F32, isOutput=False)
    mk_d = nc.declare_dram_parameter("masks", [16, 128, QB], BF, isOutput=False)
    out_d = nc.declare_dram_parameter("out", [S, HL], F32, isOutput=True)

    xt_r = xt_d[:].rearrange("(i p) s -> p i s", p=128)
    wk_r = wk_d[:].rearrange("(i p) n -> p i n", p=128)
    wv_r = wv_d[:].rearrange("(i p) n -> p i n", p=128)
    wq_r = wq_d[:].rearrange("(i p) n -> p i n", p=128)
    wp_r = wp_d[:].rearrange("(i p) n -> p i n", p=128)
    mk_r = mk_d[:].rearrange("k p q -> p k q")

    with tile.TileContext(nc) as tc:
        with (
            tc.tile_pool(name="persist", bufs=1) as PP,
            tc.tile_pool(name="wstream", bufs=2) as WP,
            tc.tile_pool(name="psum", bufs=1, space="PSUM") as PS,
            tc.tile_pool(name="outp", bufs=2) as OP,
            tc.tile_pool(name="dram", bufs=1, space="DRAM") as DP,
        ):
            # persistent tensors
            kt_sb = PP.tile([128, 4, S], BF, tag="kt")         # K^T head pairs
            qt_sb = PP.tile([128, 4, S], BF, tag="qt")         # Q^T head pairs
            v_sb = PP.tile([128, 16, 8, D + 1], BF, tag="v")   # V + ones col
            ot_own = PP.tile([128, 4, S], BF, tag="oto")       # own O^T
            ot_all = PP.tile([128, CK, S], BF, tag="ota")      # gathered O^T
            mask_sb = PP.tile([128, 16, QB], BF, tag="mask")
            bb_sb = PP.tile([128, HL], F32, tag="bb")
            bp1_sb = PP.tile([1, HL], F32, tag="bp1")
            wp_sb = PP.tile([128, CK, HL], BF, tag="wp")

            in_b = [DP.tile([128, 4, 512], BF, tag="ccin", bufs=4,
                            name=f"ib{qg}") for qg in range(3)]
            out_b = [DP.tile([2, 128, 4, 512], BF, tag="ccout", bufs=4,
                             name=f"ob{qg}") for qg in range(3)]
            in_h = [DP.tile([128, 2, 512], BF, tag="ccinh", bufs=2,
                            name=f"ih{j}") for j in range(2)]
            out_h = [DP.tile([2, 128, 2, 512], BF, tag="ccouth", bufs=2,
                             name=f"oh{j}") for j in range(2)]

            # ones column of V at col D (softmax denominator via PV matmul)
            for tt in range(16):
                nc.gpsimd.memset(v_sb[:, tt, :, D : D + 1], 1.0)
            nc.sync.dma_start(bp1_sb[:], bp_d[:])
            nc.gpsimd.partition_broadcast(bb_sb[:], bp1_sb[:])

            with tc.tile_pool(name="xin", bufs=1) as XP:
                xt_sb = XP.tile([128, CK, S], BF, tag="xt")

                # ---- input DMAs (gpsimd queue, ordered by first use) ----
                wk_sb = WP.tile([128, CK, HL], BF, tag="wk", bufs=1, name="wk")
                wq_sb = WP.tile([128, CK, HL], BF, tag="wq", bufs=1, name="wq")
                wv_sb = WP.tile([128, CK, HL], BF, tag="wv", bufs=1, name="wv")
                nc.gpsimd.dma_start(wk_sb[:, 0:4, :], wk_r[:, 0:4, :])
                nc.gpsimd.dma_start(xt_sb[:, 0:4, 0:512], xt_r[:, 0:4, 0:512])
                nc.gpsimd.dma_start(wk_sb[:, 4:8, :], wk_r[:, 4:8, :])
                nc.gpsimd.dma_start(xt_sb[:, 4:8, 0:512], xt_r[:, 4:8, 0:512])
                nc.gpsimd.dma_start(wq_sb[:], wq_r)
                nc.gpsimd.dma_start(wv_sb[:], wv_r)
                for nt in range(1, 4):
                    sl = slice(nt * 512, nt * 512 + 512)
                    nc.gpsimd.dma_start(xt_sb[:, :, sl], xt_r[:, :, sl])
                nc.gpsimd.dma_start(mask_sb[:], mk_r)
                nc.gpsimd.dma_start(wp_sb[:], wp_r)

                # PE warm-up while the first DMAs land
                warm = XP.tile([128, 512], BF, tag="warm")
                nc.vector.memset(warm[:], 0.0)
                wps = PS.tile([128, 512], F32, tag="proj", bufs=2, name="warmps")
                for _ in range(56):
                    nc.tensor.matmul(wps[:], warm[:, 0:128], warm[:],
                                     start=True, stop=True)

                # ---- projection units ----
                def kq_unit(which, nt, hp):
                    w_sb, dst = ((wk_sb, kt_sb) if which == "k"
                                 else (wq_sb, qt_sb))
                    sl = slice(nt * 512, nt * 512 + 512)
                    ps = PS.tile([128, 512], F32, tag="proj", bufs=2,
                                 name="pskq")
                    for c in range(CK):
                        nc.tensor.matmul(
                            ps[:],
                            w_sb[:, c, hp * 128 : hp * 128 + 128],
                            xt_sb[:, c, sl],
                            start=(c == 0),
                            stop=(c == CK - 1),
                        )
                    if which == "k":
                        nc.scalar.copy(dst[:, hp, sl], ps[:])
                    else:
                        nc.vector.tensor_copy(dst[:, hp, sl], ps[:])

                def v_unit(tt):
                    ps = PS.tile([128, 512], F32, tag="proj", bufs=2, name="psv")
                    for c in range(CK):
                        nc.tensor.matmul(
                            ps[:],
                            xt_sb[:, c, tt * 128 : tt * 128 + 128],
                            wv_sb[:, c, :],
                            start=(c == 0),
                            stop=(c == CK - 1),
                        )
                    nc.vector.tensor_copy(
                        v_sb[:, tt, :, 0:D],
                        ps[:].rearrange("p (a b) -> p a b", b=D),
                    )

                # upfront: everything qg0 needs
                for hp in range(4):
                    kq_unit("k", 0, hp)
                for hp in range(4):
                    kq_unit("q", 0, hp)
                for tt in range(4):
                    v_unit(tt)

                # ------------- P2 + interleaved fillers -------------
                with (
                    tc.tile_pool(name="ptp", bufs=4) as PTP,
                    tc.tile_pool(name="smallp", bufs=2) as SMP,
                ):
                    state = {}

                    def emit_scores(h, qg, g):
                        hp, hr = h // 2, (h % 2) * 64
                        shared = g < 2 * qg + 1
                        ps = PS.tile([128, 2, 512], F32, tag="pss", bufs=2,
                                     name="pss")
                        pt = PTP.tile([128, 2, 512], BF, tag="pt")
                        qsl = slice(qg * 512, qg * 512 + 512)
                        qsl_b = slice(qg * 512 + 256, qg * 512 + 512)
                        for i in range(2):
                            kt = 2 * g + i
                            ksl = slice(kt * 128, kt * 128 + 128)
                            if shared:
                                nc.tensor.matmul(
                                    ps[:, i, :],
                                    kt_sb[hr : hr + 64, hp, ksl],
                                    qt_sb[hr : hr + 64, hp, qsl],
                                    start=True, stop=True,
                                )
                            else:
                                nc.tensor.matmul(
                                    ps[:, i, QB:512],
                                    kt_sb[hr : hr + 64, hp, ksl],
                                    qt_sb[hr : hr + 64, hp, qsl_b],
                                    start=True, stop=True,
                                )
                        if shared:
                            nc.scalar.activation(pt[:], ps[:], EXP,
                                                 scale=float(D) ** -0.5)
                        else:
                            nc.scalar.activation(pt[:, :, QB:512], ps[:, :, QB:512],
                                                 EXP, scale=float(D) ** -0.5)
                        # causal masks for the two boundary items
                        if g >= 2 * qg:
                            coff = (g - 2 * qg) * QB
                            nc.vector.tensor_mul(
                                pt[:, :, coff : coff + QB],
                                pt[:, :, coff : coff + QB],
                                mask_sb[:, 2 * g : 2 * g + 2, :],
                            )
                        return pt

                    def emit_pv(h, qg, g, pt):
                        hp, hr = h // 2, (h % 2) * 64
                        krun_a = 4 * qg + 2
                        krun_b = 4 * qg + 4
                        if g == 0:
                            state[(h, qg)] = PS.tile(
                                [128, 512], F32, tag="pso", bufs=2,
                                name=f"po{h}_{qg}"
                            )
                        po = state[(h, qg)]
                        for i in range(2):
                            kt = 2 * g + i
                            if kt < krun_a:
                                nc.tensor.matmul(
                                    po[0:65, :], v_sb[:, kt, h, :], pt[:, i, :],
                                    start=(kt == 0), stop=(kt == krun_b - 1),
                                    skip_group_check=True,
                                )
                            else:
                                nc.tensor.matmul(
                                    po[0:65, QB:512], v_sb[:, kt, h, :],
                                    pt[:, i, QB:512],
                                    start=False, stop=(kt == krun_b - 1),
                                    skip_group_check=True,
                                )
                        if 2 * g + 1 == krun_b - 1:
                            rc = SMP.tile([128, 512], F32, tag="recip")
                            nc.vector.tensor_copy(rc[0:1, :], po[64:65, :])
                            rc2 = SMP.tile([128, 512], F32, tag="recip2")
                            nc.vector.reciprocal_approx_fast(rc2[0:1, :],
                                                             rc[0:1, :])
                            rb = SMP.tile([128, 512], F32, tag="rbc")
                            nc.gpsimd.partition_broadcast(rb[0:64, :], rc2[0:1, :])
                            qsl = slice(qg * 512, qg * 512 + 512)
                            dst = (
                                ot_own[0:64, hp, qsl]
                                if h % 2 == 0
                                else ot_own[64:128, hp, qsl]
                            )
                            nc.vector.tensor_mul(dst, po[0:64, :], rb[0:64, :])
                            del state[(h, qg)]

                    def emit_xch(qg, half=None):
                        """half=None: full 4-hp exchange; half=0/1: hp pair."""
                        qsl = slice(qg * 512, qg * 512 + 512)
                        if half is None:
                            hsl, idx = slice(0, 4), qg
                            ib, ob_ = in_b[qg], out_b[qg]
                        else:
                            hsl = slice(2 * half, 2 * half + 2)
                            ib, ob_ = in_h[half], out_h[half]
                        nc.sync.dma_start(ib[:], ot_own[:, hsl, qsl])
                        nc.gpsimd.collective_compute(
                            "AllGather",
                            mybir.AluOpType.bypass,
                            replica_groups=[[0, 1], [2, 3], [4, 5], [6, 7]],
                            ins=[ib.opt()],
                            outs=[ob_.opt()],
                        )
                        nhp = 4 if half is None else 2
                        off = 0 if half is None else 2 * half
                        for r in range(2):
                            nc.sync.dma_start(
                                ot_all[:, 4 * r + off : 4 * r + off + nhp, qsl],
                                ob_[r],
                            )

                    def emit_p3(qt):
                        qsl = slice(qt * 128, qt * 128 + 128)
                        ob = OP.tile([128, HL], F32, tag="ob", name=f"ob{qt % 4}")
                        ps = PS.tile([128, 512], F32, tag="proj", bufs=2,
                                     name="psf")
                        for hdc in range(CK):
                            nc.tensor.matmul(
                                ps[:],
                                ot_all[:, hdc, qsl],
                                wp_sb[:, hdc, :],
                                start=(hdc == 0),
                                stop=(hdc == CK - 1),
                            )
                        nc.vector.tensor_add(ob[:], ps[:], bb_sb[:])
                        nc.sync.dma_start(out_d[qsl, :], ob[:])

                    def do_fill(kind, args):
                        if kind == "kq":
                            kq_unit(*args)
                        elif kind == "v":
                            v_unit(args)
                        elif kind == "xch":
                            emit_xch(*args)
                        else:
                            emit_p3(args)

                    # fill units: (earliest_item, kind, args)
                    fills = []
                    # qg0 window (items 0..15): K nt1, Q nt1, V tt4..7
                    for j in range(4):
                        fills.append((1 + 3 * j, "kq", ("k", 1, j)))
                        fills.append((2 + 3 * j, "kq", ("q", 1, j)))
                        fills.append((3 + 3 * j, "v", 4 + j))
                    # qg1 window (16..47): xch0, K nt2, Q nt2, V tt8..11
                    fills.append((21, "xch", (0,)))
                    for j in range(4):
                        fills.append((17 + 8 * j, "kq", ("k", 2, j)))
                        fills.append((20 + 8 * j, "kq", ("q", 2, j)))
                        fills.append((23 + 8 * j, "v", 8 + j))
                    # qg2 window (48..95): xch1, K nt3, Q nt3, V tt12..15
                    fills.append((53, "xch", (1,)))
                    for j in range(4):
                        fills.append((49 + 12 * j, "kq", ("k", 3, j)))
                        fills.append((53 + 12 * j, "kq", ("q", 3, j)))
                        fills.append((57 + 12 * j, "v", 12 + j))
                    # qg3 window (96..159): xch2, then all deferred P3 work
                    # (P3 is the only PE filler dense enough to hide the
                    #  ACT-exp backlog of the biggest q-group)
                    fills.append((101, "xch", (2,)))
                    for j in range(10):
                        fills.append((102 + 5 * j, "p3", j))
                    # first half-exchange of qg3 once heads 0-3 are flushed
                    fills.append((133, "xch", (3, 0)))
                    fills.sort(key=lambda f: f[0])

                    items = [(h, qg, g) for qg in range(4) for h in range(8)
                             for g in range(2 * qg + 2)]

                    pend = []
                    for n, it in enumerate(items):
                        pt = emit_scores(*it)
                        pend.append((it, pt))
                        if len(pend) > 3:
                            old = pend.pop(0)
                            emit_pv(*old[0], old[1])
                        while fills and fills[0][0] <= n:
                            _, kind, args = fills.pop(0)
                            do_fill(kind, args)
                    for old in pend:
                        emit_pv(*old[0], old[1])
                    for _, kind, args in fills:
                        do_fill(kind, args)

                    # -------- tail: second qg3 half-exchange + last P3 ------
                    emit_xch(3, 1)
                    for qt in (10, 11):   # qg2 tiles kept back to cover the
                        emit_p3(qt)       # half-exchange latency
                    for qt in range(12, 16):
                        emit_p3(qt)

    nc.finalize()
    return nc


def _get_runner():
    """Compile once; return fn(in_maps) -> list[dict] using a cached jax jit."""
    if "runner" in _CACHE:
        return _CACHE["runner"]
    import jax
    import concourse.mybir as mybir
    from concourse import bass2jax as b2j
    from jax.experimental.shard_map import shard_map
    from jax.sharding import Mesh, PartitionSpec

    nc = _build_nc()
    b2j.install_neuronx_cc_hook()

    partition_name = nc.partition_id_tensor.name if nc.partition_id_tensor else None
    in_names, out_names, out_avals, zero_outs = [], [], [], []
    for alloc in nc.m.functions[0].allocations:
        if not isinstance(alloc, mybir.MemoryLocationSet):
            continue
        name = alloc.memorylocations[0].name
        if alloc.kind == "ExternalInput":
            if name != partition_name:
                in_names.append(name)
        elif alloc.kind == "ExternalOutput":
            shape = tuple(alloc.tensor_shape)
            dtype = mybir.dt.np(alloc.dtype)
            out_names.append(name)
            out_avals.append(jax.core.ShapedArray(shape, dtype))
            zero_outs.append(np.zeros(shape, dtype))
    n_params = len(in_names)
    n_outs = len(out_avals)
    in_names = in_names + out_names
    if partition_name is not None:
        in_names.append(partition_name)
    donate = tuple(range(n_params, n_params + n_outs))

    def _body(*args):
        operands = list(args)
        if partition_name is not None:
            operands.append(b2j.partition_id_tensor())
        outs = b2j._bass_exec_p.bind(
            *operands,
            out_avals=tuple(out_avals),
            in_names=tuple(in_names),
            out_names=tuple(out_names),
            lowering_input_output_aliases=(),
            sim_require_finite=True,
            sim_require_nnan=True,
            nc=nc,
        )
        return tuple(outs)

    try:
        devices = jax.devices("axon")[:NCORES]
    except RuntimeError:
        devices = jax.devices()[:NCORES]
    mesh = Mesh(np.asarray(devices), ("core",))
    in_specs = (PartitionSpec("core"),) * (n_params + n_outs)
    out_specs = (PartitionSpec("core"),) * n_outs
    sharded = jax.jit(
        shard_map(_body, mesh=mesh, in_specs=in_specs, out_specs=out_specs,
                  check_rep=False),
        donate_argnums=donate,
        keep_unused=True,
    )

    def runner(in_maps):
        per_core = [[np.asarray(m[nm]) for nm in in_names[:n_params]] for m in in_maps]
        concat_in = [
            np.concatenate([per_core[c][i] for c in range(NCORES)], axis=0)
            for i in range(n_params)
        ]
        concat_zeros = [
            np.zeros((NCORES * z.shape[0], *z.shape[1:]), z.dtype) for z in zero_outs
        ]
        out_arrs = sharded(*concat_in, *concat_zeros)
        return [
            {
                nm: np.asarray(out_arrs[i]).reshape(NCORES, *out_avals[i].shape)[c]
                for i, nm in enumerate(out_names)
            }
            for c in range(NCORES)
        ]

    _CACHE["nc"] = nc
    _CACHE["runner"] = runner
    return runner


def make_in_maps(x, Wq, Wk, Wv, Wp, bp):
    import ml_dtypes

    bf16 = ml_dtypes.bfloat16
    x = np.asarray(x, np.float32)
    wq_f = np.asarray(Wq, np.float32).transpose(1, 0, 2).reshape(C, HD)
    wk_f = np.asarray(Wk, np.float32).transpose(1, 0, 2).reshape(C, HD)
    wv_f = np.asarray(Wv, np.float32).transpose(1, 0, 2).reshape(C, HD)
    wp_f = np.asarray(Wp, np.float32)
    bp1 = np.asarray(bp, np.float32).reshape(1, C)

    # causal masks for boundary key tiles: mask m = 2p+j covers key tile
    # kt = 2p+j against q block p (rows 256p..256p+256)
    mk = np.zeros((16, 128, QB), np.float32)
    for p in range(8):
        qabs = p * QB + np.arange(QB)[None, :]
        for j in range(2):
            kt = 2 * p + j
            kabs = kt * 128 + np.arange(128)[:, None]
            mk[2 * p + j] = (kabs <= qabs).astype(np.float32)
    mk = mk.astype(bf16)

    xt_b = [np.ascontiguousarray(x[b].T).astype(bf16) for b in range(B)]
    in_maps = []
    for core in range(NCORES):
        b, hh = core // 2, core % 2
        csl = slice(hh * HL, hh * HL + HL)
        in_maps.append({
            "xt": xt_b[b],
            "wq": np.ascontiguousarray(wq_f[:, csl]).astype(bf16),
            "wk": np.ascontiguousarray(wk_f[:, csl]).astype(bf16),
            "wv": np.ascontiguousarray(wv_f[:, csl]).astype(bf16),
            "wp": np.ascontiguousarray(wp_f[:, csl]).astype(bf16),
            "bp": np.ascontiguousarray(bp1[:, csl]),
            "masks": mk,
        })
    return in_maps, None


def assemble(results, _unused=None):
    out = np.empty((B, S, C), np.float32)
    for core in range(NCORES):
        b, hh = core // 2, core % 2
        out[b, :, hh * HL : hh * HL + HL] = results[core]["out"]
    return out


def kernel(x, Wq, Wk, Wv, Wp, bp):
    in_maps, aux = make_in_maps(x, Wq, Wk, Wv, Wp, bp)
    runner = _get_runner()
    results = runner(in_maps)
    return assemble(results, aux)


# revision 17
# speedup vs baseline: 1.1876x; 1.1876x over previous
"""Multi-head causal attention (B=4, S=2048, C=1024, H=16, D=64) on 8 trn2 cores.

Sharding: batch x head-half. Core c = (batch b = c//2, head-half hh = c%2).
Each core projects Q/K/V for its 8 heads over the full 2048-token sequence
(no duplicated K/V work), runs causal attention for those heads, and after
each 512-row q-group exchanges the per-head attention outputs O^T with its
pair core via a 2-core AllGather (DRAM bounce). The output projection is
column-split: each core multiplies the full 16-head O^T by its 512-column
slice of Wp, so no partial-sum reduction is needed. The host concatenates
the two column halves per batch.

Per-core pipeline (matmuls bf16, fp32 PSUM):
  P1: DMA bf16 inputs; project K^T [512,2048], Q^T [512,2048] (head-pair
      partition layout), V [seq,8,65] with a ones column per head.
  P2: per q-group qg (512 rows = blocks 2qg,2qg+1), head h, kt-pair g:
      scores S^T via K^T-lhsT matmul, exp on ACT (scale 1/8), causal masks
      on DVE for boundary key tiles, PV accumulation with [V_h | ones]
      giving O^T rows + softmax denominators, reciprocal+normalize.
      After each qg: AllGather O^T slice with pair core.
  P3: per 128-q tile: out[:, own 512 cols] = O^T_all.T @ Wp_cols + bp_cols.
  K/Q/V projection chunks for later q-groups and P3 tiles are interleaved
  into P2 as PE filler work.
"""

import numpy as np

B, S, C, H, D = 4, 2048, 1024, 16, 64
HD = H * D
HL = 512           # head-local hd per core (8 heads x 64)
QB = 256           # q block width
CK = C // 128      # contraction chunks
NCORES = 8

_CACHE = {}


def _build_nc():
    import concourse.bacc as bacc
    import concourse.mybir as mybir
    import concourse.tile as tile

    dt = mybir.dt
    F32, BF = dt.float32, dt.bfloat16
    EXP = mybir.ActivationFunctionType.Exp

    nc = bacc.Bacc(num_swdge_queues=4, num_devices=NCORES)
    xt_d = nc.declare_dram_parameter("xt", [C, S], BF, isOutput=False)
    wk_d = nc.declare_dram_parameter("wk", [C, HL], BF, isOutput=False)
    wv_d = nc.declare_dram_parameter("wv", [C, HL], BF, isOutput=False)
    wq_d = nc.declare_dram_parameter("wq", [C, HL], BF, isOutput=False)
    wp_d = nc.declare_dram_parameter("wp", [HD, HL], BF, isOutput=False)
    bp_d = nc.declare_dram_parameter("bp", [1, HL], F32, isOutput=False)
    mk_d = nc.declare_dram_parameter("masks", [16, 128, QB], BF, isOutput=False)
    out_d = nc.declare_dram_parameter("out", [S, HL], F32, isOutput=True)

    xt_r = xt_d[:].rearrange("(i p) s -> p i s", p=128)
    wk_r = wk_d[:].rearrange("(i p) n -> p i n", p=128)
    wv_r = wv_d[:].rearrange("(i p) n -> p i n", p=128)
    wq_r = wq_d[:].rearrange("(i p) n -> p i n", p=128)
    wp_r = wp_d[:].rearrange("(i p) n -> p i n", p=128)
    mk_r = mk_d[:].rearrange("k p q -> p k q")

    with tile.TileContext(nc) as tc:
        with (
            tc.tile_pool(name="persist", bufs=1) as PP,
            tc.tile_pool(name="wstream", bufs=2) as WP,
            tc.tile_pool(name="psum", bufs=1, space="PSUM") as PS,
            tc.tile_pool(name="outp", bufs=2) as OP,
            tc.tile_pool(name="dram", bufs=1, space="DRAM") as DP,
        ):
            # persistent tensors
            kt_sb = PP.tile([128, 4, S], BF, tag="kt")         # K^T head pairs
            qt_sb = PP.tile([128, 4, S], BF, tag="qt")         # Q^T head pairs
            v_sb = PP.tile([128, 16, 8, D + 1], BF, tag="v")   # V + ones col
            ot_own = PP.tile([128, 4, S], BF, tag="oto")       # own O^T
            ot_all = PP.tile([128, CK, 1536], BF, tag="ota")   # gathered qg0-2
            ot_al3 = PP.tile([128, CK, 512], BF, tag="ota3")   # gathered qg3
            # (separate tile so tail P3 on qg2 can't false-dep on the
            #  last exchange's writes)
            mask_sb = PP.tile([128, 16, QB], BF, tag="mask")
            bb_sb = PP.tile([128, HL], F32, tag="bb")
            bp1_sb = PP.tile([1, HL], F32, tag="bp1")
            wp_sb = PP.tile([128, CK, HL], BF, tag="wp")

            in_b = [DP.tile([128, 4, 512], BF, tag="ccin", bufs=4,
                            name=f"ib{qg}") for qg in range(4)]
            out_b = [DP.tile([2, 128, 4, 512], BF, tag="ccout", bufs=4,
                             name=f"ob{qg}") for qg in range(4)]

            # ones column of V at col D (softmax denominator via PV matmul)
            for tt in range(16):
                nc.gpsimd.memset(v_sb[:, tt, :, D : D + 1], 1.0)
            nc.sync.dma_start(bp1_sb[:], bp_d[:])
            nc.gpsimd.partition_broadcast(bb_sb[:], bp1_sb[:])

            with tc.tile_pool(name="xin", bufs=1) as XP:
                xt_sb = XP.tile([128, CK, S], BF, tag="xt")

                # ---- input DMAs (gpsimd queue, ordered by first use) ----
                wk_sb = WP.tile([128, CK, HL], BF, tag="wk", bufs=1, name="wk")
                wq_sb = WP.tile([128, CK, HL], BF, tag="wq", bufs=1, name="wq")
                wv_sb = WP.tile([128, CK, HL], BF, tag="wv", bufs=1, name="wv")
                nc.gpsimd.dma_start(wk_sb[:, 0:4, :], wk_r[:, 0:4, :])
                nc.gpsimd.dma_start(xt_sb[:, 0:4, 0:512], xt_r[:, 0:4, 0:512])
                nc.gpsimd.dma_start(wk_sb[:, 4:8, :], wk_r[:, 4:8, :])
                nc.gpsimd.dma_start(xt_sb[:, 4:8, 0:512], xt_r[:, 4:8, 0:512])
                nc.gpsimd.dma_start(wq_sb[:], wq_r)
                nc.gpsimd.dma_start(wv_sb[:], wv_r)
                for nt in range(1, 4):
                    sl = slice(nt * 512, nt * 512 + 512)
                    nc.gpsimd.dma_start(xt_sb[:, :, sl], xt_r[:, :, sl])
                nc.gpsimd.dma_start(mask_sb[:], mk_r)
                nc.gpsimd.dma_start(wp_sb[:], wp_r)

                # PE warm-up while the first DMAs land
                warm = XP.tile([128, 512], BF, tag="warm")
                nc.vector.memset(warm[:], 0.0)
                wps = PS.tile([128, 512], F32, tag="proj", bufs=2, name="warmps")
                for _ in range(56):
                    nc.tensor.matmul(wps[:], warm[:, 0:128], warm[:],
                                     start=True, stop=True)

                # ---- projection units ----
                def kq_unit(which, nt, hp):
                    w_sb, dst = ((wk_sb, kt_sb) if which == "k"
                                 else (wq_sb, qt_sb))
                    sl = slice(nt * 512, nt * 512 + 512)
                    ps = PS.tile([128, 512], F32, tag="proj", bufs=2,
                                 name="pskq")
                    for c in range(CK):
                        nc.tensor.matmul(
                            ps[:],
                            w_sb[:, c, hp * 128 : hp * 128 + 128],
                            xt_sb[:, c, sl],
                            start=(c == 0),
                            stop=(c == CK - 1),
                        )
                    if which == "k":
                        nc.scalar.copy(dst[:, hp, sl], ps[:])
                    else:
                        nc.vector.tensor_copy(dst[:, hp, sl], ps[:])

                def v_unit(tt):
                    ps = PS.tile([128, 512], F32, tag="proj", bufs=2, name="psv")
                    for c in range(CK):
                        nc.tensor.matmul(
                            ps[:],
                            xt_sb[:, c, tt * 128 : tt * 128 + 128],
                            wv_sb[:, c, :],
                            start=(c == 0),
                            stop=(c == CK - 1),
                        )
                    nc.vector.tensor_copy(
                        v_sb[:, tt, :, 0:D],
                        ps[:].rearrange("p (a b) -> p a b", b=D),
                    )

                # upfront: everything qg0 needs
                for hp in range(4):
                    kq_unit("k", 0, hp)
                for hp in range(4):
                    kq_unit("q", 0, hp)
                for tt in range(4):
                    v_unit(tt)

                # ------------- P2 + interleaved fillers -------------
                with (
                    tc.tile_pool(name="ptp", bufs=4) as PTP,
                    tc.tile_pool(name="smallp", bufs=2) as SMP,
                ):
                    state = {}

                    def emit_scores(h, qg, g):
                        hp, hr = h // 2, (h % 2) * 64
                        shared = g < 2 * qg + 1
                        ps = PS.tile([128, 2, 512], F32, tag="pss", bufs=2,
                                     name="pss")
                        pt = PTP.tile([128, 2, 512], BF, tag="pt")
                        qsl = slice(qg * 512, qg * 512 + 512)
                        qsl_b = slice(qg * 512 + 256, qg * 512 + 512)
                        for i in range(2):
                            kt = 2 * g + i
                            ksl = slice(kt * 128, kt * 128 + 128)
                            if shared:
                                nc.tensor.matmul(
                                    ps[:, i, :],
                                    kt_sb[hr : hr + 64, hp, ksl],
                                    qt_sb[hr : hr + 64, hp, qsl],
                                    start=True, stop=True,
                                )
                            else:
                                nc.tensor.matmul(
                                    ps[:, i, QB:512],
                                    kt_sb[hr : hr + 64, hp, ksl],
                                    qt_sb[hr : hr + 64, hp, qsl_b],
                                    start=True, stop=True,
                                )
                        if shared:
                            nc.scalar.activation(pt[:], ps[:], EXP,
                                                 scale=float(D) ** -0.5)
                        else:
                            nc.scalar.activation(pt[:, :, QB:512], ps[:, :, QB:512],
                                                 EXP, scale=float(D) ** -0.5)
                        # causal masks for the two boundary items
                        if g >= 2 * qg:
                            coff = (g - 2 * qg) * QB
                            nc.vector.tensor_mul(
                                pt[:, :, coff : coff + QB],
                                pt[:, :, coff : coff + QB],
                                mask_sb[:, 2 * g : 2 * g + 2, :],
                            )
                        return pt

                    def emit_pv(h, qg, g, pt):
                        hp, hr = h // 2, (h % 2) * 64
                        krun_a = 4 * qg + 2
                        krun_b = 4 * qg + 4
                        if g == 0:
                            state[(h, qg)] = PS.tile(
                                [128, 512], F32, tag="pso", bufs=2,
                                name=f"po{h}_{qg}"
                            )
                        po = state[(h, qg)]
                        for i in range(2):
                            kt = 2 * g + i
                            if kt < krun_a:
                                nc.tensor.matmul(
                                    po[0:65, :], v_sb[:, kt, h, :], pt[:, i, :],
                                    start=(kt == 0), stop=(kt == krun_b - 1),
                                    skip_group_check=True,
                                )
                            else:
                                nc.tensor.matmul(
                                    po[0:65, QB:512], v_sb[:, kt, h, :],
                                    pt[:, i, QB:512],
                                    start=False, stop=(kt == krun_b - 1),
                                    skip_group_check=True,
                                )
                        if 2 * g + 1 == krun_b - 1:
                            rc = SMP.tile([128, 512], F32, tag="recip")
                            nc.vector.tensor_copy(rc[0:1, :], po[64:65, :])
                            rc2 = SMP.tile([128, 512], F32, tag="recip2")
                            nc.vector.reciprocal_approx_fast(rc2[0:1, :],
                                                             rc[0:1, :])
                            rb = SMP.tile([128, 512], F32, tag="rbc")
                            nc.gpsimd.partition_broadcast(rb[0:64, :], rc2[0:1, :])
                            qsl = slice(qg * 512, qg * 512 + 512)
                            dst = (
                                ot_own[0:64, hp, qsl]
                                if h % 2 == 0
                                else ot_own[64:128, hp, qsl]
                            )
                            nc.vector.tensor_mul(dst, po[0:64, :], rb[0:64, :])
                            del state[(h, qg)]

                    def emit_xch(qg):
                        qsl = slice(qg * 512, qg * 512 + 512)
                        nc.sync.dma_start(in_b[qg][:], ot_own[:, :, qsl])
                        nc.gpsimd.collective_compute(
                            "AllGather",
                            mybir.AluOpType.bypass,
                            replica_groups=[[0, 1], [2, 3], [4, 5], [6, 7]],
                            ins=[in_b[qg].opt()],
                            outs=[out_b[qg].opt()],
                        )
                        dst = ot_al3 if qg == 3 else ot_all
                        dsl = slice(0, 512) if qg == 3 else qsl
                        for r in range(2):
                            nc.sync.dma_start(
                                dst[:, 4 * r : 4 * r + 4, dsl],
                                out_b[qg][r],
                            )

                    def emit_p3(qt):
                        qsl = slice(qt * 128, qt * 128 + 128)
                        src = ot_al3 if qt >= 12 else ot_all
                        ssl = (slice((qt - 12) * 128, (qt - 12) * 128 + 128)
                               if qt >= 12 else qsl)
                        ob = OP.tile([128, HL], F32, tag="ob", name=f"obp{qt % 4}")
                        ps = PS.tile([128, 512], F32, tag="proj", bufs=2,
                                     name="psf")
                        for hdc in range(CK):
                            nc.tensor.matmul(
                                ps[:],
                                src[:, hdc, ssl],
                                wp_sb[:, hdc, :],
                                start=(hdc == 0),
                                stop=(hdc == CK - 1),
                            )
                        nc.vector.tensor_add(ob[:], ps[:], bb_sb[:])
                        nc.sync.dma_start(out_d[qsl, :], ob[:])

                    def do_fill(kind, args):
                        if kind == "kq":
                            kq_unit(*args)
                        elif kind == "v":
                            v_unit(args)
                        elif kind == "xch":
                            emit_xch(args)
                        else:
                            emit_p3(args)

                    # fill units: (earliest_item, kind, args).
                    # P3 fills are concentrated in the qg3 window (the only
                    # filler dense enough to hide the ACT-exp backlog there),
                    # and each P3 is emitted either well after or before the
                    # nearest exchange so coarse ot_all dep tracking cannot
                    # stall the PE on an in-flight collective.
                    fills = []
                    # qg0 window (items 0..15): K nt1, Q nt1, V tt4..7
                    for j in range(4):
                        fills.append((1 + 3 * j, "kq", ("k", 1, j)))
                        fills.append((2 + 3 * j, "kq", ("q", 1, j)))
                        fills.append((3 + 3 * j, "v", 4 + j))
                    # qg1 window (16..47): xch0, K nt2, Q nt2, V tt8..11
                    fills.append((21, "xch", 0))
                    for j in range(4):
                        fills.append((17 + 8 * j, "kq", ("k", 2, j)))
                        fills.append((20 + 8 * j, "kq", ("q", 2, j)))
                        fills.append((23 + 8 * j, "v", 8 + j))
                    # qg2 window (48..95): xch1, K nt3, Q nt3, V tt12..15
                    fills.append((53, "xch", 1))
                    for j in range(4):
                        fills.append((49 + 12 * j, "kq", ("k", 3, j)))
                        fills.append((53 + 12 * j, "kq", ("q", 3, j)))
                        fills.append((57 + 12 * j, "v", 12 + j))
                    # qg3 window (96..159): P3 qg0 tiles first (their data is
                    # long since gathered), then xch2, then P3 qg1 tiles far
                    # enough after xch2 that its collective has landed.
                    for j in range(4):
                        fills.append((97 + 3 * j, "p3", j))
                    fills.append((109, "xch", 2))
                    for j in range(4):
                        fills.append((124 + 6 * j, "p3", 4 + j))
                    fills.sort(key=lambda f: f[0])

                    items = [(h, qg, g) for qg in range(4) for h in range(8)
                             for g in range(2 * qg + 2)]

                    pend = []
                    for n, it in enumerate(items):
                        pt = emit_scores(*it)
                        pend.append((it, pt))
                        if len(pend) > 3:
                            old = pend.pop(0)
                            emit_pv(*old[0], old[1])
                        while fills and fills[0][0] <= n:
                            _, kind, args = fills.pop(0)
                            do_fill(kind, args)
                    for old in pend:
                        emit_pv(*old[0], old[1])
                    for _, kind, args in fills:
                        do_fill(kind, args)

                    # ---- tail: last exchange hidden behind P3 qg2 tiles ----
                    emit_xch(3)
                    for qt in range(8, 12):   # qg2 tiles (read ot_all, not
                        emit_p3(qt)           # ot_al3: run during collective)
                    for qt in range(12, 16):
                        emit_p3(qt)

    nc.finalize()
    return nc


def _get_runner():
    """Compile once; return fn(in_maps) -> list[dict] using a cached jax jit."""
    if "runner" in _CACHE:
        return _CACHE["runner"]
    import jax
    import concourse.mybir as mybir
    from concourse import bass2jax as b2j
    from jax.experimental.shard_map import shard_map
    from jax.sharding import Mesh, PartitionSpec

    nc = _build_nc()
    b2j.install_neuronx_cc_hook()

    partition_name = nc.partition_id_tensor.name if nc.partition_id_tensor else None
    in_names, out_names, out_avals, zero_outs = [], [], [], []
    for alloc in nc.m.functions[0].allocations:
        if not isinstance(alloc, mybir.MemoryLocationSet):
            continue
        name = alloc.memorylocations[0].name
        if alloc.kind == "ExternalInput":
            if name != partition_name:
                in_names.append(name)
        elif alloc.kind == "ExternalOutput":
            shape = tuple(alloc.tensor_shape)
            dtype = mybir.dt.np(alloc.dtype)
            out_names.append(name)
            out_avals.append(jax.core.ShapedArray(shape, dtype))
            zero_outs.append(np.zeros(shape, dtype))
    n_params = len(in_names)
    n_outs = len(out_avals)
    in_names = in_names + out_names
    if partition_name is not None:
        in_names.append(partition_name)
    donate = tuple(range(n_params, n_params + n_outs))

    def _body(*args):
        operands = list(args)
        if partition_name is not None:
            operands.append(b2j.partition_id_tensor())
        outs = b2j._bass_exec_p.bind(
            *operands,
            out_avals=tuple(out_avals),
            in_names=tuple(in_names),
            out_names=tuple(out_names),
            lowering_input_output_aliases=(),
            sim_require_finite=True,
            sim_require_nnan=True,
            nc=nc,
        )
        return tuple(outs)

    try:
        devices = jax.devices("axon")[:NCORES]
    except RuntimeError:
        devices = jax.devices()[:NCORES]
    mesh = Mesh(np.asarray(devices), ("core",))
    in_specs = (PartitionSpec("core"),) * (n_params + n_outs)
    out_specs = (PartitionSpec("core"),) * n_outs
    sharded = jax.jit(
        shard_map(_body, mesh=mesh, in_specs=in_specs, out_specs=out_specs,
                  check_rep=False),
        donate_argnums=donate,
        keep_unused=True,
    )

    def runner(in_maps):
        per_core = [[np.asarray(m[nm]) for nm in in_names[:n_params]] for m in in_maps]
        concat_in = [
            np.concatenate([per_core[c][i] for c in range(NCORES)], axis=0)
            for i in range(n_params)
        ]
        concat_zeros = [
            np.zeros((NCORES * z.shape[0], *z.shape[1:]), z.dtype) for z in zero_outs
        ]
        out_arrs = sharded(*concat_in, *concat_zeros)
        return [
            {
                nm: np.asarray(out_arrs[i]).reshape(NCORES, *out_avals[i].shape)[c]
                for i, nm in enumerate(out_names)
            }
            for c in range(NCORES)
        ]

    _CACHE["nc"] = nc
    _CACHE["runner"] = runner
    return runner


def make_in_maps(x, Wq, Wk, Wv, Wp, bp):
    import ml_dtypes

    bf16 = ml_dtypes.bfloat16
    x = np.asarray(x, np.float32)
    wq_f = np.asarray(Wq, np.float32).transpose(1, 0, 2).reshape(C, HD)
    wk_f = np.asarray(Wk, np.float32).transpose(1, 0, 2).reshape(C, HD)
    wv_f = np.asarray(Wv, np.float32).transpose(1, 0, 2).reshape(C, HD)
    wp_f = np.asarray(Wp, np.float32)
    bp1 = np.asarray(bp, np.float32).reshape(1, C)

    # causal masks for boundary key tiles: mask m = 2p+j covers key tile
    # kt = 2p+j against q block p (rows 256p..256p+256)
    mk = np.zeros((16, 128, QB), np.float32)
    for p in range(8):
        qabs = p * QB + np.arange(QB)[None, :]
        for j in range(2):
            kt = 2 * p + j
            kabs = kt * 128 + np.arange(128)[:, None]
            mk[2 * p + j] = (kabs <= qabs).astype(np.float32)
    mk = mk.astype(bf16)

    xt_b = [np.ascontiguousarray(x[b].T).astype(bf16) for b in range(B)]
    in_maps = []
    for core in range(NCORES):
        b, hh = core // 2, core % 2
        csl = slice(hh * HL, hh * HL + HL)
        in_maps.append({
            "xt": xt_b[b],
            "wq": np.ascontiguousarray(wq_f[:, csl]).astype(bf16),
            "wk": np.ascontiguousarray(wk_f[:, csl]).astype(bf16),
            "wv": np.ascontiguousarray(wv_f[:, csl]).astype(bf16),
            "wp": np.ascontiguousarray(wp_f[:, csl]).astype(bf16),
            "bp": np.ascontiguousarray(bp1[:, csl]),
            "masks": mk,
        })
    return in_maps, None


def assemble(results, _unused=None):
    out = np.empty((B, S, C), np.float32)
    for core in range(NCORES):
        b, hh = core // 2, core % 2
        out[b, :, hh * HL : hh * HL + HL] = results[core]["out"]
    return out


def kernel(x, Wq, Wk, Wv, Wp, bp):
    in_maps, aux = make_in_maps(x, Wq, Wk, Wv, Wp, bp)
    runner = _get_runner()
    results = runner(in_maps)
    return assemble(results, aux)


# revision 21
# speedup vs baseline: 1.1987x; 1.0093x over previous
"""Multi-head causal attention (B=4, S=2048, C=1024, H=16, D=64) on 8 trn2 cores.

Sharding: batch x head-half. Core c = (batch b = c//2, head-half hh = c%2).
Each core projects Q/K/V for its 8 heads over the full 2048-token sequence
(no duplicated K/V work), runs causal attention for those heads, and after
each 512-row q-group exchanges the per-head attention outputs O^T with its
pair core via a 2-core AllGather (DRAM bounce). The output projection is
column-split: each core multiplies the full 16-head O^T by its 512-column
slice of Wp, so no partial-sum reduction is needed. The host concatenates
the two column halves per batch.

Per-core pipeline (matmuls bf16, fp32 PSUM):
  P1: DMA bf16 inputs; project K^T [512,2048], Q^T [512,2048] (head-pair
      partition layout), V [seq,8,65] with a ones column per head.
  P2: per q-group qg (512 rows = blocks 2qg,2qg+1), head h, kt-pair g:
      scores S^T via K^T-lhsT matmul, exp on ACT (scale 1/8), causal masks
      on DVE for boundary key tiles, PV accumulation with [V_h | ones]
      giving O^T rows + softmax denominators, reciprocal+normalize.
      After each qg: AllGather O^T slice with pair core.
  P3: per 128-q tile: out[:, own 512 cols] = O^T_all.T @ Wp_cols + bp_cols.
  K/Q/V projection chunks for later q-groups and P3 tiles are interleaved
  into P2 as PE filler work.
"""

import numpy as np

B, S, C, H, D = 4, 2048, 1024, 16, 64
HD = H * D
HL = 512           # head-local hd per core (8 heads x 64)
QB = 256           # q block width
CK = C // 128      # contraction chunks
NCORES = 8

_CACHE = {}


def _build_nc():
    import concourse.bacc as bacc
    import concourse.mybir as mybir
    import concourse.tile as tile

    dt = mybir.dt
    F32, BF = dt.float32, dt.bfloat16
    EXP = mybir.ActivationFunctionType.Exp

    nc = bacc.Bacc(num_swdge_queues=4, num_devices=NCORES)
    xt_d = nc.declare_dram_parameter("xt", [C, S], BF, isOutput=False)
    wk_d = nc.declare_dram_parameter("wk", [C, HL], BF, isOutput=False)
    wv_d = nc.declare_dram_parameter("wv", [C, HL], BF, isOutput=False)
    wq_d = nc.declare_dram_parameter("wq", [C, HL], BF, isOutput=False)
    wp_d = nc.declare_dram_parameter("wp", [HD, HL], BF, isOutput=False)
    bp_d = nc.declare_dram_parameter("bp", [1, HL], F32, isOutput=False)
    mk_d = nc.declare_dram_parameter("masks", [16, 128, QB], BF, isOutput=False)
    out_d = nc.declare_dram_parameter("out", [S, HL], F32, isOutput=True)

    xt_r = xt_d[:].rearrange("(i p) s -> p i s", p=128)
    wk_r = wk_d[:].rearrange("(i p) n -> p i n", p=128)
    wv_r = wv_d[:].rearrange("(i p) n -> p i n", p=128)
    wq_r = wq_d[:].rearrange("(i p) n -> p i n", p=128)
    wp_r = wp_d[:].rearrange("(i p) n -> p i n", p=128)
    mk_r = mk_d[:].rearrange("k p q -> p k q")

    with tile.TileContext(nc) as tc:
        with (
            tc.tile_pool(name="persist", bufs=1) as PP,
            tc.tile_pool(name="wstream", bufs=2) as WP,
            tc.tile_pool(name="psum", bufs=1, space="PSUM") as PS,
            tc.tile_pool(name="outp", bufs=2) as OP,
            tc.tile_pool(name="dram", bufs=1, space="DRAM") as DP,
        ):
            # persistent tensors
            kt_sb = PP.tile([128, 4, S], BF, tag="kt")         # K^T head pairs
            qt_sb = PP.tile([128, 4, S], BF, tag="qt")         # Q^T head pairs
            v_sb = PP.tile([128, 16, 8, D + 1], BF, tag="v")   # V + ones col
            ot_own = PP.tile([128, 4, S], BF, tag="oto")       # own O^T
            ot_all = PP.tile([128, CK, 1536], BF, tag="ota")   # gathered qg0-2
            ot_al3 = PP.tile([128, CK, 512], BF, tag="ota3")   # gathered qg3
            # (separate tile so tail P3 on qg2 can't false-dep on the
            #  last exchange's writes)
            mask_sb = PP.tile([128, 16, QB], BF, tag="mask")
            bb_sb = PP.tile([128, HL], F32, tag="bb")
            bp1_sb = PP.tile([1, HL], F32, tag="bp1")
            wp_sb = PP.tile([128, CK, HL], BF, tag="wp")

            in_b = [DP.tile([128, 4, 512], BF, tag="ccin", bufs=4,
                            name=f"ib{qg}") for qg in range(4)]
            out_b = [DP.tile([2, 128, 4, 512], BF, tag="ccout", bufs=4,
                             name=f"ob{qg}") for qg in range(4)]

            with tc.tile_pool(name="xin", bufs=1) as XP:
                xt_sb = XP.tile([128, CK, S], BF, tag="xt")

                # ---- input DMAs (gpsimd queue, ordered by first use).
                # Only the K-projection inputs are emitted up front so the
                # first 2MB gets the full DMA bandwidth; everything else is
                # emitted after the first projection units.
                wk_sb = WP.tile([128, CK, HL], BF, tag="wk", bufs=1, name="wk")
                wq_sb = WP.tile([128, CK, HL], BF, tag="wq", bufs=1, name="wq")
                wv_sb = WP.tile([128, CK, HL], BF, tag="wv", bufs=1, name="wv")
                nc.gpsimd.dma_start(wk_sb[:, 0:4, :], wk_r[:, 0:4, :])
                nc.gpsimd.dma_start(xt_sb[:, 0:4, 0:512], xt_r[:, 0:4, 0:512])
                nc.gpsimd.dma_start(wk_sb[:, 4:8, :], wk_r[:, 4:8, :])
                nc.gpsimd.dma_start(xt_sb[:, 4:8, 0:512], xt_r[:, 4:8, 0:512])

                # PE warm-up while the first DMAs land
                warm = XP.tile([128, 512], BF, tag="warm")
                nc.vector.memset(warm[:], 0.0)
                wps = PS.tile([128, 512], F32, tag="proj", bufs=2, name="warmps")
                for _ in range(48):
                    nc.tensor.matmul(wps[:], warm[:, 0:128], warm[:],
                                     start=True, stop=True)

                # ---- projection units ----
                def kq_unit(which, nt, hp):
                    w_sb, dst = ((wk_sb, kt_sb) if which == "k"
                                 else (wq_sb, qt_sb))
                    sl = slice(nt * 512, nt * 512 + 512)
                    ps = PS.tile([128, 512], F32, tag="proj", bufs=2,
                                 name="pskq")
                    for c in range(CK):
                        nc.tensor.matmul(
                            ps[:],
                            w_sb[:, c, hp * 128 : hp * 128 + 128],
                            xt_sb[:, c, sl],
                            start=(c == 0),
                            stop=(c == CK - 1),
                        )
                    if which == "k":
                        nc.scalar.copy(dst[:, hp, sl], ps[:])
                    else:
                        nc.vector.tensor_copy(dst[:, hp, sl], ps[:])

                def v_unit(tt):
                    ps = PS.tile([128, 512], F32, tag="proj", bufs=2, name="psv")
                    for c in range(CK):
                        nc.tensor.matmul(
                            ps[:],
                            xt_sb[:, c, tt * 128 : tt * 128 + 128],
                            wv_sb[:, c, :],
                            start=(c == 0),
                            stop=(c == CK - 1),
                        )
                    nc.vector.tensor_copy(
                        v_sb[:, tt, :, 0:D],
                        ps[:].rearrange("p (a b) -> p a b", b=D),
                    )

                # upfront: everything qg0 needs, with the remaining input
                # DMAs emitted between projection units in priority order
                for hp in range(2):
                    kq_unit("k", 0, hp)
                nc.gpsimd.dma_start(wq_sb[:], wq_r)
                for hp in range(2, 4):
                    kq_unit("k", 0, hp)
                nc.gpsimd.dma_start(wv_sb[:], wv_r)
                for hp in range(2):
                    kq_unit("q", 0, hp)
                for nt in range(1, 4):
                    sl = slice(nt * 512, nt * 512 + 512)
                    nc.gpsimd.dma_start(xt_sb[:, :, sl], xt_r[:, :, sl])
                for hp in range(2, 4):
                    kq_unit("q", 0, hp)
                nc.gpsimd.dma_start(mask_sb[:], mk_r)
                nc.gpsimd.dma_start(wp_sb[:], wp_r)
                # ones column of V (softmax denominator via PV matmul) and
                # the broadcast bias, deferred off the critical DMA window
                nc.gpsimd.memset(v_sb[:, :, :, D : D + 1], 1.0)
                nc.sync.dma_start(bp1_sb[:], bp_d[:])
                nc.gpsimd.partition_broadcast(bb_sb[:], bp1_sb[:])
                for tt in range(4):
                    v_unit(tt)

                # ------------- P2 + interleaved fillers -------------
                with (
                    tc.tile_pool(name="ptp", bufs=6) as PTP,
                    tc.tile_pool(name="smallp", bufs=1) as SMP,
                ):
                    state = {}

                    def emit_scores(h, qg, g):
                        hp, hr = h // 2, (h % 2) * 64
                        shared = g < 2 * qg + 1
                        ps = PS.tile([128, 2, 512], F32, tag="pss", bufs=2,
                                     name="pss")
                        pt = PTP.tile([128, 2, 512], BF, tag="pt")
                        qsl = slice(qg * 512, qg * 512 + 512)
                        qsl_b = slice(qg * 512 + 256, qg * 512 + 512)
                        for i in range(2):
                            kt = 2 * g + i
                            ksl = slice(kt * 128, kt * 128 + 128)
                            if shared:
                                nc.tensor.matmul(
                                    ps[:, i, :],
                                    kt_sb[hr : hr + 64, hp, ksl],
                                    qt_sb[hr : hr + 64, hp, qsl],
                                    start=True, stop=True,
                                )
                            else:
                                nc.tensor.matmul(
                                    ps[:, i, QB:512],
                                    kt_sb[hr : hr + 64, hp, ksl],
                                    qt_sb[hr : hr + 64, hp, qsl_b],
                                    start=True, stop=True,
                                )
                        if shared:
                            nc.scalar.activation(pt[:], ps[:], EXP,
                                                 scale=float(D) ** -0.5)
                        else:
                            nc.scalar.activation(pt[:, :, QB:512], ps[:, :, QB:512],
                                                 EXP, scale=float(D) ** -0.5)
                        # causal masks for the two boundary items
                        if g >= 2 * qg:
                            coff = (g - 2 * qg) * QB
                            nc.vector.tensor_mul(
                                pt[:, :, coff : coff + QB],
                                pt[:, :, coff : coff + QB],
                                mask_sb[:, 2 * g : 2 * g + 2, :],
                            )
                        return pt

                    def emit_pv(h, qg, g, pt):
                        hp, hr = h // 2, (h % 2) * 64
                        krun_a = 4 * qg + 2
                        krun_b = 4 * qg + 4
                        if g == 0:
                            state[(h, qg)] = PS.tile(
                                [128, 512], F32, tag="pso", bufs=2,
                                name=f"po{h}_{qg}"
                            )
                        po = state[(h, qg)]
                        for i in range(2):
                            kt = 2 * g + i
                            if kt < krun_a:
                                nc.tensor.matmul(
                                    po[0:65, :], v_sb[:, kt, h, :], pt[:, i, :],
                                    start=(kt == 0), stop=(kt == krun_b - 1),
                                    skip_group_check=True,
                                )
                            else:
                                nc.tensor.matmul(
                                    po[0:65, QB:512], v_sb[:, kt, h, :],
                                    pt[:, i, QB:512],
                                    start=False, stop=(kt == krun_b - 1),
                                    skip_group_check=True,
                                )
                        if 2 * g + 1 == krun_b - 1:
                            rc = SMP.tile([128, 512], F32, tag="recip")
                            nc.vector.tensor_copy(rc[0:1, :], po[64:65, :])
                            rc2 = SMP.tile([128, 512], F32, tag="recip2")
                            nc.vector.reciprocal_approx_fast(rc2[0:1, :],
                                                             rc[0:1, :])
                            rb = SMP.tile([128, 512], F32, tag="rbc")
                            nc.gpsimd.partition_broadcast(rb[0:64, :], rc2[0:1, :])
                            qsl = slice(qg * 512, qg * 512 + 512)
                            dst = (
                                ot_own[0:64, hp, qsl]
                                if h % 2 == 0
                                else ot_own[64:128, hp, qsl]
                            )
                            nc.vector.tensor_mul(dst, po[0:64, :], rb[0:64, :])
                            del state[(h, qg)]

                    def emit_xch(qg):
                        qsl = slice(qg * 512, qg * 512 + 512)
                        if qg == 3:
                            # per-hp-pair stores: earlier heads' O^T ships
                            # while the last heads are still normalizing
                            for hp in range(4):
                                nc.sync.dma_start(
                                    in_b[qg][:, hp, :],
                                    ot_own[:, hp, qsl],
                                )
                        else:
                            nc.sync.dma_start(in_b[qg][:], ot_own[:, :, qsl])
                        nc.gpsimd.collective_compute(
                            "AllGather",
                            mybir.AluOpType.bypass,
                            replica_groups=[[0, 1], [2, 3], [4, 5], [6, 7]],
                            ins=[in_b[qg].opt()],
                            outs=[out_b[qg].opt()],
                        )
                        dst = ot_al3 if qg == 3 else ot_all
                        dsl = slice(0, 512) if qg == 3 else qsl
                        for r in range(2):
                            nc.sync.dma_start(
                                dst[:, 4 * r : 4 * r + 4, dsl],
                                out_b[qg][r],
                            )

                    def emit_p3(qt):
                        qsl = slice(qt * 128, qt * 128 + 128)
                        src = ot_al3 if qt >= 12 else ot_all
                        ssl = (slice((qt - 12) * 128, (qt - 12) * 128 + 128)
                               if qt >= 12 else qsl)
                        ob = OP.tile([128, HL], F32, tag="ob", name=f"obp{qt % 4}")
                        ps = PS.tile([128, 512], F32, tag="proj", bufs=2,
                                     name="psf")
                        for hdc in range(CK):
                            nc.tensor.matmul(
                                ps[:],
                                src[:, hdc, ssl],
                                wp_sb[:, hdc, :],
                                start=(hdc == 0),
                                stop=(hdc == CK - 1),
                            )
                        nc.vector.tensor_add(ob[:], ps[:], bb_sb[:])
                        nc.sync.dma_start(out_d[qsl, :], ob[:])

                    def do_fill(kind, args):
                        if kind == "kq":
                            kq_unit(*args)
                        elif kind == "v":
                            v_unit(args)
                        elif kind == "xch":
                            emit_xch(args)
                        else:
                            emit_p3(args)

                    # fill units: (earliest_item, kind, args).
                    # P3 fills are concentrated in the qg3 window (the only
                    # filler dense enough to hide the ACT-exp backlog there),
                    # and each P3 is emitted either well after or before the
                    # nearest exchange so coarse ot_all dep tracking cannot
                    # stall the PE on an in-flight collective.
                    fills = []
                    # qg0 window (items 0..15): K nt1, Q nt1, V tt4..7
                    for j in range(4):
                        fills.append((1 + 3 * j, "kq", ("k", 1, j)))
                        fills.append((2 + 3 * j, "kq", ("q", 1, j)))
                        fills.append((3 + 3 * j, "v", 4 + j))
                    # qg1 window (16..47): xch0, K nt2, Q nt2, V tt8..11
                    fills.append((21, "xch", 0))
                    for j in range(4):
                        fills.append((17 + 8 * j, "kq", ("k", 2, j)))
                        fills.append((20 + 8 * j, "kq", ("q", 2, j)))
                        fills.append((23 + 8 * j, "v", 8 + j))
                    # qg2 window (48..95): xch1, K nt3, Q nt3, V tt12..15
                    fills.append((53, "xch", 1))
                    for j in range(4):
                        fills.append((49 + 12 * j, "kq", ("k", 3, j)))
                        fills.append((53 + 12 * j, "kq", ("q", 3, j)))
                        fills.append((57 + 12 * j, "v", 12 + j))
                    # qg3 window (96..159): P3 qg0 tiles first (their data is
                    # long since gathered), then xch2, then P3 qg1 tiles far
                    # enough after xch2 that its collective has landed.
                    for j in range(4):
                        fills.append((97 + 3 * j, "p3", j))
                    fills.append((109, "xch", 2))
                    for j in range(4):
                        fills.append((124 + 6 * j, "p3", 4 + j))
                    fills.sort(key=lambda f: f[0])

                    items = [(h, qg, g) for qg in range(4) for h in range(8)
                             for g in range(2 * qg + 2)]

                    pend = []
                    for n, it in enumerate(items):
                        pt = emit_scores(*it)
                        pend.append((it, pt))
                        if len(pend) > 3:
                            old = pend.pop(0)
                            emit_pv(*old[0], old[1])
                        while fills and fills[0][0] <= n:
                            _, kind, args = fills.pop(0)
                            do_fill(kind, args)
                    for old in pend:
                        emit_pv(*old[0], old[1])
                    for _, kind, args in fills:
                        do_fill(kind, args)

                    # ---- tail: last exchange hidden behind P3 qg2 tiles ----
                    emit_xch(3)
                    for qt in range(8, 12):   # qg2 tiles (read ot_all, not
                        emit_p3(qt)           # ot_al3: run during collective)
                    for qt in range(12, 16):
                        emit_p3(qt)

    nc.finalize()
    return nc


def _get_runner():
    """Compile once; return fn(in_maps) -> list[dict] using a cached jax jit."""
    if "runner" in _CACHE:
        return _CACHE["runner"]
    import jax
    import concourse.mybir as mybir
    from concourse import bass2jax as b2j
    from jax.experimental.shard_map import shard_map
    from jax.sharding import Mesh, PartitionSpec

    nc = _build_nc()
    b2j.install_neuronx_cc_hook()

    partition_name = nc.partition_id_tensor.name if nc.partition_id_tensor else None
    in_names, out_names, out_avals, zero_outs = [], [], [], []
    for alloc in nc.m.functions[0].allocations:
        if not isinstance(alloc, mybir.MemoryLocationSet):
            continue
        name = alloc.memorylocations[0].name
        if alloc.kind == "ExternalInput":
            if name != partition_name:
                in_names.append(name)
        elif alloc.kind == "ExternalOutput":
            shape = tuple(alloc.tensor_shape)
            dtype = mybir.dt.np(alloc.dtype)
            out_names.append(name)
            out_avals.append(jax.core.ShapedArray(shape, dtype))
            zero_outs.append(np.zeros(shape, dtype))
    n_params = len(in_names)
    n_outs = len(out_avals)
    in_names = in_names + out_names
    if partition_name is not None:
        in_names.append(partition_name)
    donate = tuple(range(n_params, n_params + n_outs))

    def _body(*args):
        operands = list(args)
        if partition_name is not None:
            operands.append(b2j.partition_id_tensor())
        outs = b2j._bass_exec_p.bind(
            *operands,
            out_avals=tuple(out_avals),
            in_names=tuple(in_names),
            out_names=tuple(out_names),
            lowering_input_output_aliases=(),
            sim_require_finite=True,
            sim_require_nnan=True,
            nc=nc,
        )
        return tuple(outs)

    try:
        devices = jax.devices("axon")[:NCORES]
    except RuntimeError:
        devices = jax.devices()[:NCORES]
    mesh = Mesh(np.asarray(devices), ("core",))
    in_specs = (PartitionSpec("core"),) * (n_params + n_outs)
    out_specs = (PartitionSpec("core"),) * n_outs
    sharded = jax.jit(
        shard_map(_body, mesh=mesh, in_specs=in_specs, out_specs=out_specs,
                  check_rep=False),
        donate_argnums=donate,
        keep_unused=True,
    )

    def runner(in_maps):
        per_core = [[np.asarray(m[nm]) for nm in in_names[:n_params]] for m in in_maps]
        concat_in = [
            np.concatenate([per_core[c][i] for c in range(NCORES)], axis=0)
            for i in range(n_params)
        ]
        concat_zeros = [
            np.zeros((NCORES * z.shape[0], *z.shape[1:]), z.dtype) for z in zero_outs
        ]
        out_arrs = sharded(*concat_in, *concat_zeros)
        return [
            {
                nm: np.asarray(out_arrs[i]).reshape(NCORES, *out_avals[i].shape)[c]
                for i, nm in enumerate(out_names)
            }
            for c in range(NCORES)
        ]

    _CACHE["nc"] = nc
    _CACHE["runner"] = runner
    return runner


def make_in_maps(x, Wq, Wk, Wv, Wp, bp):
    import ml_dtypes

    bf16 = ml_dtypes.bfloat16
    x = np.asarray(x, np.float32)
    wq_f = np.asarray(Wq, np.float32).transpose(1, 0, 2).reshape(C, HD)
    wk_f = np.asarray(Wk, np.float32).transpose(1, 0, 2).reshape(C, HD)
    wv_f = np.asarray(Wv, np.float32).transpose(1, 0, 2).reshape(C, HD)
    wp_f = np.asarray(Wp, np.float32)
    bp1 = np.asarray(bp, np.float32).reshape(1, C)

    # causal masks for boundary key tiles: mask m = 2p+j covers key tile
    # kt = 2p+j against q block p (rows 256p..256p+256)
    mk = np.zeros((16, 128, QB), np.float32)
    for p in range(8):
        qabs = p * QB + np.arange(QB)[None, :]
        for j in range(2):
            kt = 2 * p + j
            kabs = kt * 128 + np.arange(128)[:, None]
            mk[2 * p + j] = (kabs <= qabs).astype(np.float32)
    mk = mk.astype(bf16)

    xt_b = [np.ascontiguousarray(x[b].T).astype(bf16) for b in range(B)]
    in_maps = []
    for core in range(NCORES):
        b, hh = core // 2, core % 2
        csl = slice(hh * HL, hh * HL + HL)
        in_maps.append({
            "xt": xt_b[b],
            "wq": np.ascontiguousarray(wq_f[:, csl]).astype(bf16),
            "wk": np.ascontiguousarray(wk_f[:, csl]).astype(bf16),
            "wv": np.ascontiguousarray(wv_f[:, csl]).astype(bf16),
            "wp": np.ascontiguousarray(wp_f[:, csl]).astype(bf16),
            "bp": np.ascontiguousarray(bp1[:, csl]),
            "masks": mk,
        })
    return in_maps, None


def assemble(results, _unused=None):
    out = np.empty((B, S, C), np.float32)
    for core in range(NCORES):
        b, hh = core // 2, core % 2
        out[b, :, hh * HL : hh * HL + HL] = results[core]["out"]
    return out


def kernel(x, Wq, Wk, Wv, Wp, bp):
    in_maps, aux = make_in_maps(x, Wq, Wk, Wv, Wp, bp)
    runner = _get_runner()
    results = runner(in_maps)
    return assemble(results, aux)
